# revision 4
# baseline (speedup 1.0000x reference)
"""Causal self-attention (B=4, T=2048, C=1024, H=16) on 8 TRN2 NeuronCores.

Sharding: core c handles batch b=c//2 and head-half hh=c%2 (8 heads).
Each core computes q/k/v projections for its heads, causal attention, and a
partial output projection (row-parallel w_proj); the host sums the two
partials per batch.

v4 design (ACT=exp-only, PE kept saturated via clocked filler interleave):
- ACT runs ONLY the softmax exp; qk psum eviction is fused into DVE
  scalar_tensor_tensor ops ((psum+bias)*cos / *sin), rotate-half is a single
  DVE stream_shuffle (head dims pre-permuted host-side so rope pairs sit
  16 rows apart inside each 32-partition block), mask-multiply moved to
  gpsimd.
- PSUM: 2x scores tiles (4 banks) + oA/oB (2 banks) + 2x 1-bank filler
  tiles = 8 banks, so projection fillers never contend with attention psum.
- A virtual PE/ACT clock pumps filler matmul granules (qkv/out projections
  of neighbouring chunks) into the PE stream exactly when the exp pipeline
  would otherwise stall it, keeping the PE array busy and at max p-state.
- attn@v uses a ones-column in v for softmax denominators; v bias and output
  bias fold to the host (softmax rows sum to 1).
"""

import sys

sys.path.insert(0, "/opt/trn_rl_repo")

from contextlib import ExitStack

import numpy as np

import concourse.bass as bass
import concourse.tile as tile
from concourse import bacc, mybir
from concourse.bass_utils import run_bass_kernel_spmd

F32 = mybir.dt.float32
F16 = mybir.dt.float16
AL = mybir.AluOpType
AF = mybir.ActivationFunctionType

B, T, C, H, HD = 4, 2048, 1024, 16, 64
NCORE = 8
HH = H // 2  # heads per core
NP = HH // 2  # head pairs per core
KC = C // 128  # contraction chunks
NT = T // 128  # 128-row time tiles
NQC = T // 512  # 512-query chunks
ROPE_THETA = 10000.0
SKEW = 5  # attnV trails scores by this many key tiles

# rope pairs (d, d+32) are hosted at rows (r, r+16) of each 32-row block
SWAP16 = list(range(16, 32)) + list(range(0, 16))

_CACHE = {}


def _build_module():
    nc = bacc.Bacc("TRN2", target_bir_lowering=False, debug=False)

    xT = nc.dram_tensor("xT", [C, T], F16, kind="ExternalInput")
    wq = nc.dram_tensor("wq", [C, 512], F16, kind="ExternalInput")
    wk = nc.dram_tensor("wk", [C, 512], F16, kind="ExternalInput")
    wv = nc.dram_tensor("wv", [C, 512], F16, kind="ExternalInput")
    wp = nc.dram_tensor("wp", [512, C], F16, kind="ExternalInput")
    bqk = nc.dram_tensor("bqk", [2, NP, 128], F32, kind="ExternalInput")
    cosr = nc.dram_tensor("cosr", [128, T], F16, kind="ExternalInput")
    sinp = nc.dram_tensor("sinp", [128, T], F16, kind="ExternalInput")
    mask = nc.dram_tensor("mask", [128, 2, 128], F16, kind="ExternalInput")
    onesc = nc.dram_tensor("onesc", [128, NT, HH, 1], F16, kind="ExternalInput")
    y = nc.dram_tensor("y", [T, C], F16, kind="ExternalOutput")

    with tile.TileContext(nc) as tc, ExitStack() as ctx:
        consts = ctx.enter_context(tc.tile_pool(name="consts", bufs=1))
        persist = ctx.enter_context(tc.tile_pool(name="persist", bufs=1))
        xp = ctx.enter_context(tc.tile_pool(name="xp", bufs=3))
        rp = ctx.enter_context(tc.tile_pool(name="rp", bufs=3))
        ptp = ctx.enter_context(tc.tile_pool(name="ptp", bufs=SKEW + 3))
        nrm = ctx.enter_context(tc.tile_pool(name="nrm", bufs=2))
        yp = ctx.enter_context(tc.tile_pool(name="yp", bufs=2))
        scp = ctx.enter_context(tc.tile_pool(name="scp", bufs=2, space="PSUM"))
        fpl = ctx.enter_context(tc.tile_pool(name="fpl", bufs=2, space="PSUM"))
        opl = ctx.enter_context(tc.tile_pool(name="opl", bufs=1, space="PSUM"))

        # ---- constants ----
        bqk_sb = consts.tile([128, 2, NP], F32)
        wq_sb = consts.tile([128, KC, 512], F16)
        wk_sb = consts.tile([128, KC, 512], F16)
        cos_sb = consts.tile([128, T], F16)
        sin_sb = consts.tile([128, T], F16)
        mask_sb = consts.tile([128, 2, 128], F16)
        wv_sb = consts.tile([128, KC, 512], F16)
        wp_sb = consts.tile([128, 4, C], F16)

        # ---- persistent activations ----
        qT = persist.tile([128, NP, T], F16)
        kT = persist.tile([128, NP, T], F16)
        vp = persist.tile([128, NT, HH, 65], F16)
        OT = persist.tile([128, NP, T], F16)

        # ---- virtual engine clocks (ns) for filler pumping ----
        clk = {"pe": 0.0, "act": 0.0}

        def mm_ns(n):
            return n * 0.4167 + 40.0

        xcs = {}

        def emit_xc(j, parts):
            xc = xp.tile([128, KC, 512], F16, tag="xc")
            nk = slice(j * 512, (j + 1) * 512)
            src = xT.rearrange("(kc p) t -> p kc t", p=128)[:, :, nk]
            for eng, k0, k1 in parts:
                for kc in range(k0, k1):
                    eng.dma_start(
                        out=xc[:, kc : kc + 1, :], in_=src[:, kc : kc + 1, :]
                    )
            xcs[j] = xc

        def emit_qkproj_half(j, p, which):
            # one 1-bank psum chain: q (which=0) or k (which=1) for pair p
            nk = slice(j * 512, (j + 1) * 512)
            xc = xcs[j]
            wsb = wq_sb if which == 0 else wk_sb
            dstT = qT if which == 0 else kT
            ps = fpl.tile([128, 512], F32, tag="f")
            for kc in range(KC):
                nc.tensor.matmul(
                    ps[:],
                    wsb[:, kc, p * 128 : (p + 1) * 128],
                    xc[:, kc, :],
                    start=(kc == 0),
                    stop=(kc == KC - 1),
                )
            clk["pe"] += KC * mm_ns(512)
            # fused eviction: (psum + bias) * cos / * sin on DVE, then
            # rotate-half via stream_shuffle and the final add
            bap = bqk_sb[:, which, p : p + 1]
            t1 = rp.tile([128, 512], F16, tag="t1")
            s1 = rp.tile([128, 512], F16, tag="s1")
            s2 = rp.tile([128, 512], F16, tag="s2")
            nc.vector.scalar_tensor_tensor(
                t1[:], ps[:], bap, cos_sb[:, nk], AL.add, AL.mult
            )
            nc.vector.scalar_tensor_tensor(
                s1[:], ps[:], bap, sin_sb[:, nk], AL.add, AL.mult
            )
            nc.vector.stream_shuffle(s2[:], s1[:], SWAP16)
            nc.vector.tensor_add(dstT[:, p, nk], t1[:], s2[:])

        def emit_vproj_block(j, blk):
            xc = xcs[j]
            ps = fpl.tile([128, 512], F32, tag="f")
            for kc in range(KC):
                nc.tensor.matmul(
                    ps[:],
                    xc[:, kc, blk * 128 : (blk + 1) * 128],
                    wv_sb[:, kc, :],
                    start=(kc == 0),
                    stop=(kc == KC - 1),
                )
            clk["pe"] += KC * mm_ns(512)
            nc.vector.tensor_copy(
                vp[:, 4 * j + blk, :, 0:64],
                ps.rearrange("p (h d) -> p h d", h=HH),
            )
            if blk == 3:
                xcs.pop(j)

        y16s = {}

        def emit_oproj_half(j, tt, nn):
            t0 = j * 512 + tt * 128
            ps = fpl.tile([128, 512], F32, tag="f")
            for kc in range(4):
                nc.tensor.matmul(
                    ps[:],
                    OT[:, kc, t0 : t0 + 128],
                    wp_sb[:, kc, nn * 512 : (nn + 1) * 512],
                    start=(kc == 0),
                    stop=(kc == 3),
                )
            clk["pe"] += 4 * mm_ns(512)
            if nn == 0:
                y16 = yp.tile([128, C], F16, tag="y16")
                y16s[tt] = y16
            y16 = y16s[tt]
            nc.vector.tensor_copy(y16[:, nn * 512 : (nn + 1) * 512], ps[:])
            if nn == 1:
                nc.sync.dma_start(out=y[t0 : t0 + 128, :], in_=y16s.pop(tt)[:])

        fillers = []

        def pump(target):
            while fillers and clk["pe"] < target:
                fillers.pop(0)()

        def emit_attention_pair(j, p):
            oA = opl.tile([65, 512], F32, tag="oA")
            oB = opl.tile([65, 512], F32, tag="oB")
            nkt = 4 * (j + 1)
            pend = []

            def attn_v(kt, pt, span, co):
                for h, o in ((0, oA), (1, oB)):
                    nc.tensor.matmul(
                        o[:, co:512],
                        vp[:, kt, p * 2 + h, :],
                        pt[:, h, 0:span],
                        start=(kt == 0),
                        stop=(kt == nkt - 1),
                    )
                clk["pe"] += 2 * mm_ns(span)

            for kt in range(nkt):
                i = kt - 4 * j
                span = 512 if i < 0 else 512 - 128 * i
                co = 512 - span
                q0 = j * 512 + co
                pump(clk["act"])
                sc = scp.tile([128, 2, 512], F32, tag="sc")
                for h in range(2):
                    nc.tensor.matmul(
                        sc[:, h, 0:span],
                        kT[h * 64 : (h + 1) * 64, p,
                           kt * 128 : (kt + 1) * 128],
                        qT[h * 64 : (h + 1) * 64, p, q0 : q0 + span],
                        start=True,
                        stop=True,
                        tile_position=(h * 64, 0),
                    )
                clk["pe"] += span * 0.4167 + 80.0
                pt = ptp.tile([128, 2, 512], F16, tag="pt")
                nc.scalar.activation(
                    pt[:, :, 0:span], sc[:, :, 0:span], AF.Exp
                )
                clk["act"] = max(clk["act"], clk["pe"]) + (
                    2 * span * 0.833 + 190.0
                )
                if i >= 0:
                    nc.gpsimd.tensor_mul(
                        pt[:, :, 0:128], pt[:, :, 0:128], mask_sb[:]
                    )
                pend.append((kt, pt, span, co))
                if len(pend) > SKEW:
                    attn_v(*pend.pop(0))
            while pend:
                attn_v(*pend.pop(0))

            # normalize: divide by the ones-column sums (psum row 64)
            jq = slice(j * 512, (j + 1) * 512)
            for h, o in ((0, oA), (1, oB)):
                dn = nrm.tile([1, 512], F32, tag=f"dn{h}")
                nc.vector.tensor_copy(dn[:], o[64:65, :])
                dr = nrm.tile([1, 512], F32, tag=f"dr{h}")
                nc.vector.reciprocal(dr[:], dn[:])
                rb = nrm.tile([64, 512], F32, tag=f"rb{h}")
                nc.gpsimd.partition_broadcast(rb[:], dr[:])
                nc.vector.tensor_mul(
                    OT[h * 64 : (h + 1) * 64, p, jq], o[0:64, :], rb[:]
                )

        # prologue: stripe the loads over the three DMA-capable queues so the
        # first matmul can start early.
        wqr = wq.rearrange("(kc p) n -> p kc n", p=128)
        for kc in range(0, KC, 2):
            nc.sync.dma_start(
                out=wq_sb[:, kc : kc + 2, :], in_=wqr[:, kc : kc + 2, :]
            )
        emit_xc(0, [(nc.scalar, 0, 4), (nc.gpsimd, 4, 8)])
        nc.scalar.dma_start(out=bqk_sb[:], in_=bqk.rearrange("a p r -> r a p"))
        wkr = wk.rearrange("(kc p) n -> p kc n", p=128)
        for kc in range(0, KC, 4):
            nc.gpsimd.dma_start(
                out=wk_sb[:, kc : kc + 4, :], in_=wkr[:, kc : kc + 4, :]
            )
        nc.scalar.dma_start(out=cos_sb[:], in_=cosr[:])
        nc.gpsimd.dma_start(out=sin_sb[:], in_=sinp[:])
        nc.sync.dma_start(out=wp_sb[:], in_=wp.rearrange("(kc r) n -> r kc n", r=128))
        nc.scalar.dma_start(out=mask_sb[:], in_=mask[:])
        nc.gpsimd.dma_start(out=wv_sb[:], in_=wv.rearrange("(kc p) n -> p kc n", p=128))
        nc.scalar.dma_start(out=vp[:, :, :, 64:65], in_=onesc[:])

        # prologue compute: chunk-0 projections
        for p in range(NP):
            emit_qkproj_half(0, p, 0)
            emit_qkproj_half(0, p, 1)
        emit_xc(1, [(nc.sync, 0, 8)])
        for blk in range(4):
            emit_vproj_block(0, blk)

        for j in range(NQC):
            if j + 2 < NQC:
                emit_xc(j + 2, [(nc.sync, 0, 8)])
            # build this chunk's filler queue: previous chunk's output
            # projection interleaved with next chunk's qk projections, then
            # next chunk's v projection
            fillers.clear()
            for tt in range(4):
                if j + 1 < NQC:
                    fillers.append(
                        lambda j=j, p=tt: emit_qkproj_half(j + 1, p, 0)
                    )
                    fillers.append(
                        lambda j=j, p=tt: emit_qkproj_half(j + 1, p, 1)
                    )
                if j > 0:
                    fillers.append(
                        lambda j=j, tt=tt: emit_oproj_half(j - 1, tt, 0)
                    )
                    fillers.append(
                        lambda j=j, tt=tt: emit_oproj_half(j - 1, tt, 1)
                    )
            if j + 1 < NQC:
                for blk in range(4):
                    fillers.append(
                        lambda j=j, blk=blk: emit_vproj_block(j + 1, blk)
                    )

            for p in range(NP):
                emit_attention_pair(j, p)
            while fillers:
                fillers.pop(0)()

        for tt in range(4):
            emit_oproj_half(NQC - 1, tt, 0)
            emit_oproj_half(NQC - 1, tt, 1)

    nc.compile()
    return nc


# row -> head-dim permutation putting rope pairs (d, d+32) at rows (r, r+16)
# inside each 32-row block, so rotate-half is a single stream_shuffle
_PI = np.concatenate(
    [np.arange(0, 16), np.arange(32, 48), np.arange(16, 32), np.arange(48, 64)]
)


def _rope_tables():
    freqs = 1.0 / (ROPE_THETA ** (np.arange(0, HD, 2, dtype=np.float32) / HD))
    ang = np.arange(T, dtype=np.float32)[:, None] * freqs[None, :]  # [T, 32]
    cos = np.cos(ang)  # [T, 32]
    sin = np.sin(ang)
    crow = cos[:, _PI % 32].T  # [64, T]
    sgn = np.where(_PI < 32, 1.0, -1.0).astype(np.float32)
    srow = (sin[:, _PI % 32] * sgn[None, :]).T
    cos_rep = np.tile(crow, (2, 1))  # [128, T]
    sin_pm = np.tile(srow, (2, 1))
    return cos_rep, sin_pm


def _f16(a):
    return np.ascontiguousarray(a).astype(np.float16)


def _prep_inputs(x, w_qkv, b_qkv, w_proj):
    cos_rep, sin_pm = _rope_tables()
    km = np.arange(128)
    mask1 = (km[:, None] <= km[None, :]).astype(np.float32)  # keep k <= q
    mask2 = np.stack([mask1, mask1], axis=1)  # [128, 2, 128]
    # per-head column permutation for the rope row layout
    colperm = np.concatenate([h * 64 + _PI for h in range(HH)])
    in_maps = []
    for c in range(NCORE):
        b, hh = c // 2, c % 2
        s = hh * 512
        m = {
            "xT": _f16(x[b].T),
            "wq": _f16(w_qkv[:, s : s + 512][:, colperm] / 8.0),
            "wk": _f16(w_qkv[:, C + s : C + s + 512][:, colperm]),
            "wv": _f16(w_qkv[:, 2 * C + s : 2 * C + s + 512]),
            "wp": _f16(w_proj[s : s + 512, :]),
            "bqk": np.stack(
                [
                    (b_qkv[s : s + 512][colperm] / 8.0).reshape(NP, 128),
                    b_qkv[C + s : C + s + 512][colperm].reshape(NP, 128),
                ]
            ).astype(np.float32),
            "onesc": np.ones((128, NT, HH, 1), np.float16),
            "cosr": _f16(cos_rep),
            "sinp": _f16(sin_pm),
            "mask": _f16(mask2),
        }
        in_maps.append(m)
    return in_maps


def _run(x, w_qkv, b_qkv, w_proj, b_proj, trace=False):
    if "nc" not in _CACHE:
        _CACHE["nc"] = _build_module()
    nc = _CACHE["nc"]
    x = np.asarray(x, np.float32)
    w_qkv = np.asarray(w_qkv, np.float32)
    b_qkv = np.asarray(b_qkv, np.float32)
    w_proj = np.asarray(w_proj, np.float32)
    b_proj = np.asarray(b_proj, np.float32)
    in_maps = _prep_inputs(x, w_qkv, b_qkv, w_proj)
    res = run_bass_kernel_spmd(nc, in_maps, core_ids=list(range(NCORE)), trace=trace)
    # host-side: sum row-parallel partials and add the folded biases
    # (attn @ (v + bv) = attn @ v + bv since softmax rows sum to 1)
    ybias = (b_qkv[2 * C :] @ w_proj + b_proj).astype(np.float32)
    out = np.empty((B, T, C), np.float32)
    for b in range(B):
        out[b] = (
            res.results[2 * b]["y"].astype(np.float32)
            + res.results[2 * b + 1]["y"].astype(np.float32)
            + ybias
        )
    return out, res


def kernel(x, w_qkv, b_qkv, w_proj, b_proj, n_heads=16):
    out, _ = _run(x, w_qkv, b_qkv, w_proj, b_proj, trace=False)
    return out


# revision 5
# speedup vs baseline: 1.0351x; 1.0351x over previous
"""Causal self-attention (B=4, T=2048, C=1024, H=16) on 8 TRN2 NeuronCores.

Sharding: core c handles batch b=c//2 and head-half hh=c%2 (8 heads).
Each core computes q/k/v projections for its heads, causal attention, and a
partial output projection (row-parallel w_proj); the host sums the two
partials per batch.

v4 design (ACT=exp-only, PE kept saturated via clocked filler interleave):
- ACT runs ONLY the softmax exp; qk psum eviction is fused into DVE
  scalar_tensor_tensor ops ((psum+bias)*cos / *sin), rotate-half is a single
  DVE stream_shuffle (head dims pre-permuted host-side so rope pairs sit
  16 rows apart inside each 32-partition block), mask-multiply moved to
  gpsimd.
- PSUM: 2x scores tiles (4 banks) + oA/oB (2 banks) + 2x 1-bank filler
  tiles = 8 banks, so projection fillers never contend with attention psum.
- A virtual PE/ACT clock pumps filler matmul granules (qkv/out projections
  of neighbouring chunks) into the PE stream exactly when the exp pipeline
  would otherwise stall it, keeping the PE array busy and at max p-state.
- attn@v uses a ones-column in v for softmax denominators; v bias and output
  bias fold to the host (softmax rows sum to 1).
"""

import sys

sys.path.insert(0, "/opt/trn_rl_repo")

from contextlib import ExitStack

import numpy as np

import concourse.bass as bass
import concourse.tile as tile
from concourse import bacc, mybir
from concourse.bass_utils import run_bass_kernel_spmd

F32 = mybir.dt.float32
F16 = mybir.dt.float16
AL = mybir.AluOpType
AF = mybir.ActivationFunctionType

B, T, C, H, HD = 4, 2048, 1024, 16, 64
NCORE = 8
HH = H // 2  # heads per core
NP = HH // 2  # head pairs per core
KC = C // 128  # contraction chunks
NT = T // 128  # 128-row time tiles
NQC = T // 512  # 512-query chunks
ROPE_THETA = 10000.0
SKEW = 5  # attnV trails scores by this many key tiles

# rope pairs (d, d+32) are hosted at rows (r, r+16) of each 32-row block
SWAP16 = list(range(16, 32)) + list(range(0, 16))

_CACHE = {}


def _build_module():
    nc = bacc.Bacc("TRN2", target_bir_lowering=False, debug=False)

    xT = nc.dram_tensor("xT", [C, T], F16, kind="ExternalInput")
    wq = nc.dram_tensor("wq", [C, 512], F16, kind="ExternalInput")
    wk = nc.dram_tensor("wk", [C, 512], F16, kind="ExternalInput")
    wv = nc.dram_tensor("wv", [C, 512], F16, kind="ExternalInput")
    wp = nc.dram_tensor("wp", [512, C], F16, kind="ExternalInput")
    bqk = nc.dram_tensor("bqk", [2, NP, 128], F32, kind="ExternalInput")
    cosr = nc.dram_tensor("cosr", [128, T], F16, kind="ExternalInput")
    sinp = nc.dram_tensor("sinp", [128, T], F16, kind="ExternalInput")
    mask = nc.dram_tensor("mask", [128, 2, 128], F16, kind="ExternalInput")
    onesc = nc.dram_tensor("onesc", [128, NT, HH, 1], F16, kind="ExternalInput")
    y = nc.dram_tensor("y", [T, C], F16, kind="ExternalOutput")

    with tile.TileContext(nc) as tc, ExitStack() as ctx:
        consts = ctx.enter_context(tc.tile_pool(name="consts", bufs=1))
        persist = ctx.enter_context(tc.tile_pool(name="persist", bufs=1))
        xp = ctx.enter_context(tc.tile_pool(name="xp", bufs=3))
        rp = ctx.enter_context(tc.tile_pool(name="rp", bufs=3))
        ptp = ctx.enter_context(tc.tile_pool(name="ptp", bufs=SKEW + 3))
        nrm = ctx.enter_context(tc.tile_pool(name="nrm", bufs=2))
        yp = ctx.enter_context(tc.tile_pool(name="yp", bufs=2))
        scp = ctx.enter_context(tc.tile_pool(name="scp", bufs=2, space="PSUM"))
        fpl = ctx.enter_context(tc.tile_pool(name="fpl", bufs=2, space="PSUM"))
        opl = ctx.enter_context(tc.tile_pool(name="opl", bufs=1, space="PSUM"))

        # ---- constants ----
        bqk_sb = consts.tile([128, 2, NP], F32)
        wq_sb = consts.tile([128, KC, 512], F16)
        wk_sb = consts.tile([128, KC, 512], F16)
        cos_sb = consts.tile([128, T], F16)
        sin_sb = consts.tile([128, T], F16)
        mask_sb = consts.tile([128, 2, 128], F16)
        wv_sb = consts.tile([128, KC, 512], F16)
        wp_sb = consts.tile([128, 4, C], F16)

        # ---- persistent activations ----
        qT = persist.tile([128, NP, T], F16)
        kT = persist.tile([128, NP, T], F16)
        vp = persist.tile([128, NT, HH, 65], F16)
        OT = persist.tile([128, NP, T], F16)

        # ---- virtual engine clocks (ns) for filler pumping ----
        clk = {"pe": 0.0, "act": 0.0}

        def mm_ns(n):
            return n * 0.4167 + 40.0

        xcs = {}

        def emit_xc(j, parts):
            xc = xp.tile([128, KC, 512], F16, tag="xc")
            nk = slice(j * 512, (j + 1) * 512)
            src = xT.rearrange("(kc p) t -> p kc t", p=128)[:, :, nk]
            for eng, k0, k1 in parts:
                for kc in range(k0, k1):
                    eng.dma_start(
                        out=xc[:, kc : kc + 1, :], in_=src[:, kc : kc + 1, :]
                    )
            xcs[j] = xc

        def emit_qkproj_half(j, p, which):
            # one 1-bank psum chain: q (which=0) or k (which=1) for pair p
            nk = slice(j * 512, (j + 1) * 512)
            xc = xcs[j]
            wsb = wq_sb if which == 0 else wk_sb
            dstT = qT if which == 0 else kT
            ps = fpl.tile([128, 512], F32, tag="f")
            for kc in range(KC):
                nc.tensor.matmul(
                    ps[:],
                    wsb[:, kc, p * 128 : (p + 1) * 128],
                    xc[:, kc, :],
                    start=(kc == 0),
                    stop=(kc == KC - 1),
                )
            clk["pe"] += KC * mm_ns(512)
            # fused eviction: (psum + bias) * cos / * sin on DVE, then
            # rotate-half via stream_shuffle and the final add
            bap = bqk_sb[:, which, p : p + 1]
            t1 = rp.tile([128, 512], F16, tag="t1")
            s1 = rp.tile([128, 512], F16, tag="s1")
            s2 = rp.tile([128, 512], F16, tag="s2")
            nc.vector.scalar_tensor_tensor(
                t1[:], ps[:], bap, cos_sb[:, nk], AL.add, AL.mult
            )
            nc.vector.scalar_tensor_tensor(
                s1[:], ps[:], bap, sin_sb[:, nk], AL.add, AL.mult
            )
            nc.vector.stream_shuffle(s2[:], s1[:], SWAP16)
            nc.vector.tensor_add(dstT[:, p, nk], t1[:], s2[:])

        def emit_vproj_block(j, blk):
            xc = xcs[j]
            ps = fpl.tile([128, 512], F32, tag="f")
            for kc in range(KC):
                nc.tensor.matmul(
                    ps[:],
                    xc[:, kc, blk * 128 : (blk + 1) * 128],
                    wv_sb[:, kc, :],
                    start=(kc == 0),
                    stop=(kc == KC - 1),
                )
            clk["pe"] += KC * mm_ns(512)
            nc.vector.tensor_copy(
                vp[:, 4 * j + blk, :, 0:64],
                ps.rearrange("p (h d) -> p h d", h=HH),
            )
            if blk == 3:
                xcs.pop(j)

        y16s = {}

        def emit_oproj_half(j, tt, nn):
            t0 = j * 512 + tt * 128
            ps = fpl.tile([128, 512], F32, tag="f")
            for kc in range(4):
                nc.tensor.matmul(
                    ps[:],
                    OT[:, kc, t0 : t0 + 128],
                    wp_sb[:, kc, nn * 512 : (nn + 1) * 512],
                    start=(kc == 0),
                    stop=(kc == 3),
                )
            clk["pe"] += 4 * mm_ns(512)
            if nn == 0:
                y16 = yp.tile([128, C], F16, tag="y16")
                y16s[tt] = y16
            y16 = y16s[tt]
            nc.vector.tensor_copy(y16[:, nn * 512 : (nn + 1) * 512], ps[:])
            if nn == 1:
                nc.sync.dma_start(out=y[t0 : t0 + 128, :], in_=y16s.pop(tt)[:])

        fillers = []

        def pump(target):
            while fillers and clk["pe"] < target:
                fillers.pop(0)()

        def emit_attention_pair(j, p):
            oA = opl.tile([65, 512], F32, tag="oA")
            oB = opl.tile([65, 512], F32, tag="oB")
            nkt = 4 * (j + 1)
            pend = []

            def attn_v(kt, pt, span, co):
                for h, o in ((0, oA), (1, oB)):
                    nc.tensor.matmul(
                        o[:, co:512],
                        vp[:, kt, p * 2 + h, :],
                        pt[:, h, 0:span],
                        start=(kt == 0),
                        stop=(kt == nkt - 1),
                    )
                clk["pe"] += 2 * mm_ns(span)

            for kt in range(nkt):
                i = kt - 4 * j
                span = 512 if i < 0 else 512 - 128 * i
                co = 512 - span
                q0 = j * 512 + co
                pump(clk["act"])
                sc = scp.tile([128, 2, 512], F32, tag="sc")
                for h in range(2):
                    nc.tensor.matmul(
                        sc[:, h, 0:span],
                        kT[h * 64 : (h + 1) * 64, p,
                           kt * 128 : (kt + 1) * 128],
                        qT[h * 64 : (h + 1) * 64, p, q0 : q0 + span],
                        start=True,
                        stop=True,
                        tile_position=(h * 64, 0),
                    )
                clk["pe"] += span * 0.4167 + 80.0
                pt = ptp.tile([128, 2, 512], F16, tag="pt")
                nc.scalar.activation(
                    pt[:, :, 0:span], sc[:, :, 0:span], AF.Exp
                )
                clk["act"] = max(clk["act"], clk["pe"]) + (
                    2 * span * 0.833 + 190.0
                )
                if i >= 0:
                    nc.gpsimd.tensor_mul(
                        pt[:, :, 0:128], pt[:, :, 0:128], mask_sb[:]
                    )
                pend.append((kt, pt, span, co))
                if len(pend) > SKEW:
                    attn_v(*pend.pop(0))
            while pend:
                attn_v(*pend.pop(0))

            # normalize: divide by the ones-column sums (psum row 64)
            jq = slice(j * 512, (j + 1) * 512)
            for h, o in ((0, oA), (1, oB)):
                dn = nrm.tile([1, 512], F32, tag=f"dn{h}")
                nc.vector.tensor_copy(dn[:], o[64:65, :])
                dd = nrm.tile([64, 8], F32, tag=f"dd{h}")
                nc.sync.dma_start(
                    out=dd[:],
                    in_=dn.rearrange("p (a b) -> p a b", a=64),
                )
                rr = nrm.tile([64, 8], F32, tag=f"rr{h}")
                nc.vector.reciprocal(rr[:], dd[:])
                dr = nrm.tile([1, 512], F32, tag=f"dr{h}")
                nc.sync.dma_start(
                    out=dr.rearrange("p (a b) -> p a b", a=64), in_=rr[:]
                )
                rb = nrm.tile([64, 512], F32, tag=f"rb{h}")
                nc.gpsimd.partition_broadcast(rb[:], dr[:])
                nc.vector.tensor_mul(
                    OT[h * 64 : (h + 1) * 64, p, jq], o[0:64, :], rb[:]
                )

        # prologue: stripe the loads over the three DMA-capable queues so the
        # first matmul can start early.
        wqr = wq.rearrange("(kc p) n -> p kc n", p=128)
        for kc in range(0, KC, 2):
            nc.sync.dma_start(
                out=wq_sb[:, kc : kc + 2, :], in_=wqr[:, kc : kc + 2, :]
            )
        emit_xc(0, [(nc.scalar, 0, 4), (nc.gpsimd, 4, 8)])
        nc.scalar.dma_start(out=bqk_sb[:], in_=bqk.rearrange("a p r -> r a p"))
        wkr = wk.rearrange("(kc p) n -> p kc n", p=128)
        for kc in range(0, KC, 4):
            nc.gpsimd.dma_start(
                out=wk_sb[:, kc : kc + 4, :], in_=wkr[:, kc : kc + 4, :]
            )
        nc.scalar.dma_start(out=cos_sb[:], in_=cosr[:])
        nc.gpsimd.dma_start(out=sin_sb[:], in_=sinp[:])
        nc.sync.dma_start(out=wp_sb[:], in_=wp.rearrange("(kc r) n -> r kc n", r=128))
        nc.scalar.dma_start(out=mask_sb[:], in_=mask[:])
        nc.gpsimd.dma_start(out=wv_sb[:], in_=wv.rearrange("(kc p) n -> p kc n", p=128))
        nc.scalar.dma_start(out=vp[:, :, :, 64:65], in_=onesc[:])

        # prologue compute: chunk-0 projections
        for p in range(NP):
            emit_qkproj_half(0, p, 0)
            emit_qkproj_half(0, p, 1)
        emit_xc(1, [(nc.sync, 0, 8)])
        for blk in range(4):
            emit_vproj_block(0, blk)

        for j in range(NQC):
            if j + 2 < NQC:
                emit_xc(j + 2, [(nc.sync, 0, 8)])
            # build this chunk's filler queue: previous chunk's output
            # projection interleaved with next chunk's qk projections, then
            # next chunk's v projection
            fillers.clear()
            for tt in range(4):
                if j + 1 < NQC:
                    fillers.append(
                        lambda j=j, p=tt: emit_qkproj_half(j + 1, p, 0)
                    )
                    fillers.append(
                        lambda j=j, p=tt: emit_qkproj_half(j + 1, p, 1)
                    )
                if j > 0:
                    fillers.append(
                        lambda j=j, tt=tt: emit_oproj_half(j - 1, tt, 0)
                    )
                    fillers.append(
                        lambda j=j, tt=tt: emit_oproj_half(j - 1, tt, 1)
                    )
            if j + 1 < NQC:
                for blk in range(4):
                    fillers.append(
                        lambda j=j, blk=blk: emit_vproj_block(j + 1, blk)
                    )

            for p in range(NP):
                emit_attention_pair(j, p)
            while fillers:
                fillers.pop(0)()

        for tt in range(4):
            emit_oproj_half(NQC - 1, tt, 0)
            emit_oproj_half(NQC - 1, tt, 1)

    nc.compile()
    return nc


# row -> head-dim permutation putting rope pairs (d, d+32) at rows (r, r+16)
# inside each 32-row block, so rotate-half is a single stream_shuffle
_PI = np.concatenate(
    [np.arange(0, 16), np.arange(32, 48), np.arange(16, 32), np.arange(48, 64)]
)


def _rope_tables():
    freqs = 1.0 / (ROPE_THETA ** (np.arange(0, HD, 2, dtype=np.float32) / HD))
    ang = np.arange(T, dtype=np.float32)[:, None] * freqs[None, :]  # [T, 32]
    cos = np.cos(ang)  # [T, 32]
    sin = np.sin(ang)
    crow = cos[:, _PI % 32].T  # [64, T]
    sgn = np.where(_PI < 32, 1.0, -1.0).astype(np.float32)
    srow = (sin[:, _PI % 32] * sgn[None, :]).T
    cos_rep = np.tile(crow, (2, 1))  # [128, T]
    sin_pm = np.tile(srow, (2, 1))
    return cos_rep, sin_pm


def _f16(a):
    return np.ascontiguousarray(a).astype(np.float16)


def _prep_inputs(x, w_qkv, b_qkv, w_proj):
    cos_rep, sin_pm = _rope_tables()
    km = np.arange(128)
    mask1 = (km[:, None] <= km[None, :]).astype(np.float32)  # keep k <= q
    mask2 = np.stack([mask1, mask1], axis=1)  # [128, 2, 128]
    # per-head column permutation for the rope row layout
    colperm = np.concatenate([h * 64 + _PI for h in range(HH)])
    in_maps = []
    for c in range(NCORE):
        b, hh = c // 2, c % 2
        s = hh * 512
        m = {
            "xT": _f16(x[b].T),
            "wq": _f16(w_qkv[:, s : s + 512][:, colperm] / 8.0),
            "wk": _f16(w_qkv[:, C + s : C + s + 512][:, colperm]),
            "wv": _f16(w_qkv[:, 2 * C + s : 2 * C + s + 512]),
            "wp": _f16(w_proj[s : s + 512, :]),
            "bqk": np.stack(
                [
                    (b_qkv[s : s + 512][colperm] / 8.0).reshape(NP, 128),
                    b_qkv[C + s : C + s + 512][colperm].reshape(NP, 128),
                ]
            ).astype(np.float32),
            "onesc": np.ones((128, NT, HH, 1), np.float16),
            "cosr": _f16(cos_rep),
            "sinp": _f16(sin_pm),
            "mask": _f16(mask2),
        }
        in_maps.append(m)
    return in_maps


def _run(x, w_qkv, b_qkv, w_proj, b_proj, trace=False):
    if "nc" not in _CACHE:
        _CACHE["nc"] = _build_module()
    nc = _CACHE["nc"]
    x = np.asarray(x, np.float32)
    w_qkv = np.asarray(w_qkv, np.float32)
    b_qkv = np.asarray(b_qkv, np.float32)
    w_proj = np.asarray(w_proj, np.float32)
    b_proj = np.asarray(b_proj, np.float32)
    in_maps = _prep_inputs(x, w_qkv, b_qkv, w_proj)
    res = run_bass_kernel_spmd(nc, in_maps, core_ids=list(range(NCORE)), trace=trace)
    # host-side: sum row-parallel partials and add the folded biases
    # (attn @ (v + bv) = attn @ v + bv since softmax rows sum to 1)
    ybias = (b_qkv[2 * C :] @ w_proj + b_proj).astype(np.float32)
    out = np.empty((B, T, C), np.float32)
    for b in range(B):
        out[b] = (
            res.results[2 * b]["y"].astype(np.float32)
            + res.results[2 * b + 1]["y"].astype(np.float32)
            + ybias
        )
    return out, res


def kernel(x, w_qkv, b_qkv, w_proj, b_proj, n_heads=16):
    out, _ = _run(x, w_qkv, b_qkv, w_proj, b_proj, trace=False)
    return out
